# revision 22
# baseline (speedup 1.0000x reference)
"""MultiHeadAttention Trainium2 Bass kernel, 8-core (batch x head-group) sharded.

Reference computation (B=4, S=2048, D=1024, H=16, d_k=64):
    Q = query @ W_q.T ; K = key @ W_k.T ; V = value @ W_v.T
    per head: attn = softmax(Q K^T / 8) @ V
    out = concat_heads(attn) @ W_o.T
Sharding: core c handles batch b = c // 2 and head-group hg = c % 2 (8 heads,
a 512-wide slice of the model dim). Core-pair partial fp16 outputs
(row-parallel W_o) are summed on the host while unsharding.

All matmul operands are fp16 (fp32 PSUM accumulation). Trace-derived budget
(per core): PE streams ~280us (scores 56us as concurrent dual-quadrant
pairs + PV 111us (M=65, the ones column rules out col-tiling) + QKV proj
83us + W_o 28us), ACT exp ~279us effective (256 ops of N=1024; PSUM
capacity forbids larger batches: sc 2x2 banks + proj 2 + ot 2 = 8). Steady
state runs at the ACT/PE joint floor (~1080ns/slot); the kernel spends its
effort on the edges: DMA-paced ramp, tiling-mode switch overhead, and the
tail.

Structure (measured 402.8us -> ~365us over this rework):
  prologue: HAM warm-up matmuls; input DMAs are chunked (<=0.5MB) across
  the sync/gpsimd rings (~120GB/s each) in score-need-order (xk before xv:
  scores gate the exp chain, V rides the PV trail); W_q/W_k ship t-major so
  head-pair slices land as 256KB units. The scalar ring carries ONLY the
  3 q-chain chunks, first: a DMA trigger whose queue-guard semaphore isn't
  ready head-of-line-blocks its engine queue, and the scalar queue carries
  the EXP chain.
  attention: 256 (qb, t, kt) score slots emitted in PAIRS (two adjacent
  64-row-mode score slots per 128-mode stretch of PV/proj work) to halve
  tiling-mode switch drains. EVERYTHING else (V projection halves, K.T
  tiles, Q.T blocks, W_o of finished blocks, softmax-normalize chains)
  flows through one deadline-tagged work queue paced by a cycle model that
  includes the exp->score semaphore latency (SEM_LAT) and switch drains
  (SW_NS), so run_spare fills the PE exactly up to each score pair's
  eligibility and the ACT chain never queues behind slack work. P.V
  matmuls trail their exp by a per-item pvq threshold (TRAIL/TRAIL_BLK/
  TRAIL_END) so block-boundary ot-ring WARs resolve off the critical path.
  tail: W_o(qb=3) partials pre-run on freed score-psum slots (two 1-bank
  accumulators per 2-bank sc slot), warm matmuls bridge the last normalize
  chain so HAM stays at 2.4GHz, and the final out-DMAs fan across all
  three rings.

Per-core dataflow (contraction always on the partition axis):
    K.T[d', s] = (W_k.T slice).T @ x_k.T    (d' on partitions)
    Q.T[d', s] likewise, projected per 512-wide q-block
    V[s, d']   = (x_v.T).T @ W_v.T          (natural layout, + ones column)
    S.T[k, q]  = (K_h.T).T @ Q_h.T          (two heads row-packed, K=64)
    expS.T     = exp(S.T / 8)               (one ACT op per k-tile, 1024 free)
    O.T+denom  = [V_h | 1].T @ expS.T       (M=65, accumulated over 16 k tiles)
    O.T norm   = O.T * (1/denom)            (DVE reciprocal + gpsimd broadcast,
                                             deferred off the block boundary)
    out[s, :]  = O.T.T @ W_o.T slice        (fp16 partial; host adds pairs)
"""
import sys

sys.path.insert(0, "/opt/trn_rl_repo")

import numpy as np

import concourse.bass as bass  # noqa: F401
import concourse.tile as tile
from concourse import bacc, mybir
from concourse.bass_utils import run_bass_kernel_spmd

F16 = mybir.dt.float16
F32 = mybir.dt.float32
EXP = mybir.ActivationFunctionType.Exp
COPY = mybir.ActivationFunctionType.Copy
MULT = mybir.AluOpType.mult

B, S, D = 4, 2048, 1024
H_PER_CORE = 8      # heads per core
DH = 64             # head dim
DP = 512            # per-core model-dim slice (8 heads x 64)
NT = 4              # d' tiles / head pairs per core
SB = 4              # 512-wide s/q blocks
KT = 16             # 128-wide k tiles
PKT = 8             # 128-wide contraction tiles for projections (D / 128)
VW = DH + 1         # V columns per head incl. ones column
N_WARM = 100        # HAM warm-up matmuls at kernel start
TRAIL = 7           # pvq items a P.V pair trails its exp (steady state)
TRAIL_BLK = 10      # trail for the first PVs of a block (ot-ring WAR room)
TRAIL_END = 5       # trail for mk_end (evacuate before the ot-ring wraps)
E_BUFS = 11         # exp-tile ring: >= TRAIL_BLK + 1

# cycle-model constants (ns) for opportunistic fill pacing
MM_NS = 219.0
ACT_NS = 1113.0
ACT_SEM = 58.0
SC_DRAIN = 170.0
SEM_LAT = 430.0     # exp-completion sem propagation to a waiting score MM
SW_NS = 90.0        # 64<->128 tiling-mode switch drain

_RUN_KWARGS = {}
_LAST_RESULT = []


def build_nc():
    nc = bacc.Bacc("TRN2", target_bir_lowering=False, debug=False)

    # activations pre-tiled on host: [sb, p, kt, 512], contiguous per partition
    xqt = nc.dram_tensor("xqt", [SB, 128, PKT, 512], F16, kind="ExternalInput")
    xkt = nc.dram_tensor("xkt", [SB, 128, PKT, 512], F16, kind="ExternalInput")
    xvt = nc.dram_tensor("xvt", [SB, 128, PKT, 512], F16, kind="ExternalInput")
    # W_q/W_k pre-tiled t-major: [p, t, kt, 128] so one head-pair's slice is
    # a 256KB contiguous chunk; W_v/W_o stay kt-major (consumed whole)
    wqt = nc.dram_tensor("wqt", [128, NT, PKT, 128], F16, kind="ExternalInput")
    wkt = nc.dram_tensor("wkt", [128, NT, PKT, 128], F16, kind="ExternalInput")
    wvt = nc.dram_tensor("wvt", [128, PKT, DP], F16, kind="ExternalInput")
    wot = nc.dram_tensor("wot", [128, NT, D], F16, kind="ExternalInput")
    out = nc.dram_tensor("out", [S, D], F16, kind="ExternalOutput")

    with tile.TileContext(nc) as tc:
        with tc.tile_pool(name="persist", bufs=1) as persist, \
             tc.tile_pool(name="psum", bufs=3, space="PSUM") as psum, \
             tc.tile_pool(name="att", bufs=2) as att:

            # ---- HAM warm-up: keep the PE busy while the first DMAs land ----
            warm_s = persist.tile([128, 128], F16)
            nc.vector.memset(warm_s[:], 1.0)
            wps = psum.tile([128, 128], F32, tag="proj", bufs=2, name="warm_ps")
            for _ in range(N_WARM):
                nc.tensor.matmul(wps[:], warm_s[:], warm_s[:],
                                 start=True, stop=True)

            # ---- persistent SBUF ----
            wk_s = persist.tile([128, NT, PKT, 128], F16)
            wv_s = persist.tile([128, PKT, DP], F16)
            wq_s = persist.tile([128, NT, PKT, 128], F16)
            wot_s = persist.tile([128, NT, D], F16)
            kt_s = persist.tile([128, NT, S], F16)          # K.T
            vext_s = persist.tile([128, KT, H_PER_CORE * VW], F16)  # [V_h | 1]
            xk_bs = [persist.tile([128, PKT, 512], F16, name=f"xk_{sb}")
                     for sb in range(SB)]

            # ones columns for the denominator rows (V part is written below)
            ones_f = persist.tile([128, KT, H_PER_CORE], F16)
            nc.vector.memset(ones_f[:], 1.0)
            nc.vector.tensor_copy(
                vext_s[:].rearrange("p k (h c) -> p k h c", c=VW)[:, :, :, DH:DH + 1],
                ones_f[:, :, :, None],
            )

            qt_tiles = {}

            # xv and xq activations share one 6-buffer tag: xq2/xq3 reuse the
            # buffers of xv0/xv1, whose V-projection reads finish ~100 slots
            # before those DMAs are triggered, so the buffer-reuse WAR waits
            # are always already satisfied (no cross-queue deadlock cycle)
            def xq_alloc(qb):
                xq_b = att.tile([128, PKT, 512], F16, tag="xbuf", bufs=6,
                                name=f"xq_{qb}")
                qt_tiles[qb] = (att.tile([128, NT, 512], F16, tag="qt", bufs=3,
                                         name=f"qt_{qb}"), xq_b)
                return xq_b

            def xq_load(qb, ring):
                xq_b = xq_alloc(qb)
                ring.dma_start(xq_b[:], xqt[qb])

            # ---- input DMAs. The scalar queue carries the EXP chain, and a
            # DMA trigger whose queue-guard semaphore isn't ready
            # head-of-line-blocks its engine's instruction queue — so the
            # scalar ring gets ONLY the 3 q-chain chunks, first, inside the
            # guard-free zone (~4 outstanding DMAs before guards appear),
            # and nothing else ever. sync and gpsimd (idle early) absorb
            # guard waits harmlessly. Chunks <=0.5MB in need-order; subtile
            # deps let each proj start as its chunk lands. ----
            xv_bs = [att.tile([128, PKT, 512], F16, tag="xbuf", bufs=6,
                              name=f"xv_{sb}") for sb in range(SB)]
            xq0_b = xq_alloc(0)
            xq1_b = xq_alloc(1)
            lo, hi = slice(0, PKT // 2), slice(PKT // 2, PKT)
            for dst, src in [(wq_s[:, 0], wqt[:, 0]),
                             (xq0_b[:, lo], xqt[0][:, lo]),
                             (xq0_b[:, hi], xqt[0][:, hi])]:
                nc.scalar.dma_start(dst, src)
            sync_chunks = [
                (wk_s[:, 0], wkt[:, 0]),            # 256KB: first kt proj
                (xk_bs[0][:, lo], xkt[0][:, lo]),
                (xk_bs[1][:, lo], xkt[1][:, lo]),   # kt(sb1) due slot 3
                (xk_bs[2][:, lo], xkt[2][:, lo]),   # scores gate the chain;
                (wv_s[:, lo], wvt[:, lo]),          # V rides the PV trail
                (xv_bs[0][:, lo], xvt[0][:, lo]),
                (xk_bs[3][:, lo], xkt[3][:, lo]),
                (wk_s[:, 1], wkt[:, 1]),            # head-pair 1 due slot 15
                (xv_bs[1][:, lo], xvt[1][:, lo]),
                (xv_bs[2][:, lo], xvt[2][:, lo]),
                (xv_bs[3][:, lo], xvt[3][:, lo]),
                (wk_s[:, 2:4], wkt[:, 2:4]),
                (xq1_b[:, lo], xqt[1][:, lo]),      # q-block 1 due slot 64
                (wot_s[:, 0:2], wot[:, 0:2]),       # W_o due ~slot 80
            ]
            gps_chunks = [
                (xk_bs[0][:, hi], xkt[0][:, hi]),
                (xk_bs[1][:, hi], xkt[1][:, hi]),
                (xk_bs[2][:, hi], xkt[2][:, hi]),
                (wv_s[:, hi], wvt[:, hi]),
                (xv_bs[0][:, hi], xvt[0][:, hi]),
                (xk_bs[3][:, hi], xkt[3][:, hi]),
                (wq_s[:, 1], wqt[:, 1]),
                (xv_bs[1][:, hi], xvt[1][:, hi]),
                (xv_bs[2][:, hi], xvt[2][:, hi]),
                (xv_bs[3][:, hi], xvt[3][:, hi]),
                (wq_s[:, 2:4], wqt[:, 2:4]),
                (xq1_b[:, hi], xqt[1][:, hi]),
                (wot_s[:, 2:4], wot[:, 2:4]),
            ]
            for dst, src in sync_chunks:
                nc.sync.dma_start(dst, src)
            for dst, src in gps_chunks:
                nc.gpsimd.dma_start(dst, src)

            # ---- work helpers (prologue + queue items) ----
            pr_ps = {}

            def ktproj(sb, t, half):
                key = ("kt", sb, t)
                if half == 0:
                    pr_ps[key] = psum.tile([128, 512], F32, tag="proj",
                                           bufs=2, name=f"psk_{sb}_{t}")
                ps = pr_ps[key]
                for kt in range(half * PKT // 2, (half + 1) * PKT // 2):
                    nc.tensor.matmul(
                        ps[:], wk_s[:, t, kt, :],
                        xk_bs[sb][:, kt, :],
                        start=kt == 0, stop=kt == PKT - 1,
                    )
                if half == 1:
                    nc.vector.tensor_copy(
                        kt_s[:, t, sb * 512:(sb + 1) * 512], ps[:])

            def vproj(st, half):
                sb, quarter = st // 4, st % 4
                key = ("v", st)
                if half == 0:
                    pr_ps[key] = psum.tile([128, 512], F32, tag="proj",
                                           bufs=2, name=f"psv_{st}")
                ps = pr_ps[key]
                for kt in range(half * PKT // 2, (half + 1) * PKT // 2):
                    nc.tensor.matmul(
                        ps[:], xv_bs[sb][:, kt, quarter * 128:(quarter + 1) * 128],
                        wv_s[:, kt, :],
                        start=kt == 0, stop=kt == PKT - 1,
                    )
                if half == 1:
                    nc.vector.tensor_copy(
                        vext_s[:, st, :].rearrange(
                            "p (h c) -> p h c", c=VW)[:, :, 0:DH],
                        ps[:].rearrange("p (h c) -> p h c", c=DH),
                    )

            def qt_proj(qb, t, half):
                qt_b, xq_b = qt_tiles[qb]
                key = ("q", qb, t)
                if half == 0:
                    pr_ps[key] = psum.tile([128, 512], F32, tag="proj",
                                           bufs=2, name=f"psq_{qb}_{t}")
                ps = pr_ps[key]
                for kt in range(half * PKT // 2, (half + 1) * PKT // 2):
                    nc.tensor.matmul(
                        ps[:], wq_s[:, t, kt, :],
                        xq_b[:, kt, :],
                        start=kt == 0, stop=kt == PKT - 1,
                    )
                if half == 1:
                    nc.vector.tensor_copy(qt_b[:, t, :], ps[:])

            ot_tiles = {}
            out_rr = [0]

            tail_dma = [False]

            def out_dma(dst, src):
                if tail_dma[0]:
                    r = (nc.sync, nc.gpsimd, nc.scalar)[out_rr[0] % 3]
                else:
                    r = nc.sync if out_rr[0] % 2 == 0 else nc.gpsimd
                r.dma_start(dst, src)
                out_rr[0] += 1

            def wo_stage(qb, si, dm, tail=False):
                ot_b = ot_tiles[qb]
                st = qb * 4 + si
                ssl = slice(si * 128, (si + 1) * 128)
                ps = psum.tile([128, 512], F32, tag="proj", bufs=2,
                               name=f"pso_{st}_{dm}")
                for t in range(NT):
                    nc.tensor.matmul(
                        ps[:], ot_b[:, t, ssl],
                        wot_s[:, t, dm * 512:(dm + 1) * 512],
                        start=t == 0, stop=t == NT - 1,
                    )
                ob = att.tile([128, 512], F16, tag="ob", bufs=5,
                              name=f"ob_{st}_{dm}")
                if tail and dm == 0:
                    # scalar engine is idle after the last EXP
                    nc.scalar.activation(ob[:], ps[:], COPY)
                else:
                    nc.vector.tensor_copy(ob[:], ps[:])
                out_dma(
                    out[st * 128:(st + 1) * 128, dm * 512:(dm + 1) * 512],
                    ob[:])

            def norm_chain(qb, t, otr_a, dn_a, otr_b, dn_b, ot_b):
                for nm, otr, dn, psl in (("a", otr_a, dn_a, slice(0, 64)),
                                         ("b", otr_b, dn_b, slice(64, 128))):
                    rd = att.tile([1, 512], F32, tag="rd", bufs=2,
                                  name=f"rd{nm}_{qb}_{t}")
                    nc.vector.reciprocal_approx_fast(rd[:], dn[:])
                    rb = att.tile([64, 512], F32, tag="rb", bufs=2,
                                  name=f"rb{nm}_{qb}_{t}")
                    nc.gpsimd.partition_broadcast(rb[:], rd[:])
                    nc.vector.tensor_tensor(
                        ot_b[psl, t, :], otr[:], rb[:], MULT)
                if (qb, t) == (SB - 1, NT - 2):
                    # ot_b[3][:, 0:3, :] is fully normalized now: the first
                    # two tail W_o partial stages can overlap the last slots
                    for si_dm in stages[:2]:
                        push(3 * MM_NS, partial(wo_partial, *si_dm),
                             dl=10**9, ms=0)

            stages = [(si, dm) for si in range(4) for dm in range(2)]
            wo_ps = {}

            def wo_partial(si, dm, tag="proj"):
                ot_b = ot_tiles[SB - 1]
                st = (SB - 1) * 4 + si
                ps = psum.tile([128, 512], F32, tag=tag, bufs=2,
                               name=f"pso_{st}_{dm}")
                wo_ps[(si, dm)] = ps
                for t in range(NT - 1):
                    nc.tensor.matmul(
                        ps[:], ot_b[:, t, si * 128:(si + 1) * 128],
                        wot_s[:, t, dm * 512:(dm + 1) * 512],
                        start=t == 0, stop=False,
                    )

            def wo_final(si, dm):
                ot_b = ot_tiles[SB - 1]
                st = (SB - 1) * 4 + si
                ps = wo_ps[(si, dm)]
                t = NT - 1
                nc.tensor.matmul(
                    ps[:], ot_b[:, t, si * 128:(si + 1) * 128],
                    wot_s[:, t, dm * 512:(dm + 1) * 512],
                    start=False, stop=True,
                )
                ob = att.tile([128, 512], F16, tag="ob", bufs=5,
                              name=f"ob_{st}_{dm}")
                if dm == 0:
                    nc.scalar.activation(ob[:], ps[:], COPY)
                else:
                    nc.vector.tensor_copy(ob[:], ps[:])
                out_dma(
                    out[st * 128:(st + 1) * 128, dm * 512:(dm + 1) * 512],
                    ob[:])

            # ---- prologue: just enough for the first score slots ----
            ktproj(0, 0, 0); ktproj(0, 0, 1)
            qt_proj(0, 0, 0); qt_proj(0, 0, 1)

            # ---- the work queue ----
            # dicts: c=PE cost ns, dl=deadline slot, ms=min slot. A projection
            # holds one sc-ring psum slot from its h0 until its h1: the "nxt"
            # link front-inserts h1 right after h0 runs, so at most one score
            # pair allocates between them and the 3-slot ring cannot wrap onto
            # the held psum.
            work = []

            def push(cost, fn, dl, ms=0, nxt=None):
                work.append({"c": cost, "dl": dl, "ms": ms, "fn": fn,
                             "nxt": nxt})

            def push2(cost, fn0, fn1, dl, ms=0):
                work.append({"c": cost, "dl": dl, "ms": ms, "fn": fn0,
                             "nxt": {"c": cost, "dl": -1e9, "ms": 0,
                                     "fn": fn1, "nxt": None}})

            from functools import partial

            for sb in range(1, SB):               # K.T head-pair 0
                push2(4 * MM_NS, partial(ktproj, sb, 0, 0),
                      partial(ktproj, sb, 0, 1), 4 * sb - 1)
            for st in range(KT):                  # V halves (DMA-ramp paced)
                push2(4 * MM_NS, partial(vproj, st, 0),
                      partial(vproj, st, 1), st + 6)
            for t in range(1, NT):                # K.T head-pairs 1-3
                for sb in range(SB):
                    push2(4 * MM_NS, partial(ktproj, sb, t, 0),
                          partial(ktproj, sb, t, 1), 16 * t + 4 * sb - 1)
                push2(4 * MM_NS, partial(qt_proj, 0, t, 0),
                      partial(qt_proj, 0, t, 1), 16 * t - 4)
            # xq2/xq3 DMAs reuse xv0/xv1 buffers: trigger them only after the
            # V projections that read those buffers have been emitted
            push(60, partial(xq_load, 2, nc.sync), 113, ms=36)
            push(60, partial(xq_load, 3, nc.sync), 177, ms=84)
            # Q.T for later q-blocks (qb 2/3 gated behind their xq_load)
            qt_ms = {1: 0, 2: 40, 3: 88}
            for qb in range(1, SB):
                for t in range(NT):
                    push2(4 * MM_NS, partial(qt_proj, qb, t, 0),
                          partial(qt_proj, qb, t, 1), 64 * qb + 16 * t - 2,
                          ms=qt_ms[qb])
            work.sort(key=lambda i: i["dl"])

            # ---- attention: 256 score slots at EXP pace, emitted in pairs ----
            pe_t, act_t = 0.0, 0.0
            exp_end = []
            pvq = []            # trailing P.V / block-end closures

            def run_due(g):
                nonlocal pe_t
                i = 0
                while i < len(work):
                    it = work[i]
                    if it["dl"] <= g + 1 and it["ms"] <= g:
                        work.pop(i)
                        it["fn"]()
                        pe_t += it["c"]
                        if it["nxt"] is not None:
                            work.insert(0, it["nxt"])
                            i = 0
                    else:
                        i += 1

            def run_spare(g):
                # fill the PE up to (but not past) the moment the NEXT pair's
                # first score becomes eligible, so that score never queues
                # behind slack work and the ACT chain never starves
                nonlocal pe_t
                tgt = (exp_end[g - 1] + SEM_LAT - 250.0) if g >= 1 else 0.0
                while work:
                    pick = None
                    for it in work:
                        if it["ms"] <= g:
                            pick = it
                            break
                    if pick is None or pe_t + pick["c"] > tgt:
                        break
                    work.remove(pick)
                    pick["fn"]()
                    pe_t += pick["c"]
                    if pick["nxt"] is not None:
                        work.insert(0, pick["nxt"])

            g = 0
            for qb in range(SB):
                if qb >= 1:                        # W_o of the previous block
                    for j, (si, dm) in enumerate(
                            (si, dm) for si in range(4) for dm in range(2)):
                        # ms must postdate the (qb-1, 3) norm chain, which
                        # trails its block end by TRAIL + dl=+8 slots
                        push(4 * MM_NS + 120,
                             partial(wo_stage, qb - 1, si, dm),
                             dl=64 * qb + 24 + 4 * j, ms=64 * qb + 16 + 2 * j)
                qt_b, _ = qt_tiles[qb]
                ot_b = att.tile([128, NT, 512], F16, tag="ot",
                                name=f"ot_{qb}")
                ot_tiles[qb] = ot_b
                for t in range(NT):
                    ota = psum.tile([65, 512], F32, tag="ot", bufs=2,
                                    name=f"ota_{qb}_{t}")
                    otb = psum.tile([65, 512], F32, tag="ot", bufs=2,
                                    name=f"otb_{qb}_{t}")
                    ha, hb = 2 * t, 2 * t + 1

                    def mk_pv(kt, e, ota=ota, otb=otb, ha=ha, hb=hb):
                        # first PVs of a block trail longer so the previous
                        # block's mk_end evacuations clear the ot ring WAR
                        def _pv():
                            nonlocal pe_t
                            nc.tensor.matmul(
                                ota[:], vext_s[:, kt, ha * VW:(ha + 1) * VW],
                                e[:, 0, :],
                                start=kt == 0, stop=kt == KT - 1,
                            )
                            nc.tensor.matmul(
                                otb[:], vext_s[:, kt, hb * VW:(hb + 1) * VW],
                                e[:, 1, :],
                                start=kt == 0, stop=kt == KT - 1,
                            )
                            pe_t += 2 * MM_NS
                        return _pv

                    def mk_end(qb=qb, t=t, ota=ota, otb=otb, ot_b=ot_b):
                        def _end():
                            if (qb, t) == (SB - 1, NT - 1):
                                # tail-critical: denominator copies + recips
                                # first (the serial chain), otr evacuations on
                                # the now-idle scalar engine in parallel
                                dns, otrs, rbs = [], [], []
                                for nm, ot_ps in (("a", ota), ("b", otb)):
                                    dn = att.tile([1, 512], F32, tag="dn",
                                                  bufs=2, name=f"dnt{nm}")
                                    nc.vector.tensor_copy(dn[:],
                                                          ot_ps[64:65, :])
                                    dns.append(dn)
                                for nm, ot_ps in (("a", ota), ("b", otb)):
                                    otr = att.tile([64, 512], F32, tag="otr",
                                                   bufs=2, name=f"otrt{nm}")
                                    nc.scalar.activation(otr[:],
                                                         ot_ps[0:64, :], COPY)
                                    otrs.append(otr)
                                for nm, dn in (("a", dns[0]), ("b", dns[1])):
                                    rd = att.tile([1, 512], F32, tag="rd",
                                                  bufs=2, name=f"rdt{nm}")
                                    nc.vector.reciprocal_approx_fast(rd[:],
                                                                     dn[:])
                                    rb = att.tile([64, 512], F32, tag="rb",
                                                  bufs=2, name=f"rbt{nm}")
                                    nc.gpsimd.partition_broadcast(rb[:],
                                                                  rd[:])
                                    rbs.append(rb)
                                nc.vector.tensor_tensor(
                                    ot_b[0:64, t, :], otrs[0][:], rbs[0][:],
                                    MULT)
                                nc.vector.tensor_tensor(
                                    ot_b[64:128, t, :], otrs[1][:], rbs[1][:],
                                    MULT)
                                return
                            evs = []
                            for nm, ot_ps in (("a", ota), ("b", otb)):
                                otr = att.tile([64, 512], F32, tag="otr",
                                               bufs=2, name=f"otr{nm}_{qb}_{t}")
                                nc.vector.tensor_copy(otr[:], ot_ps[0:64, :])
                                dn = att.tile([1, 512], F32, tag="dn", bufs=2,
                                              name=f"dn{nm}_{qb}_{t}")
                                nc.vector.tensor_copy(dn[:], ot_ps[64:65, :])
                                evs += [otr, dn]
                            push(0, partial(norm_chain, qb, t, evs[0], evs[1],
                                            evs[2], evs[3], ot_b),
                                 dl=g + 8, ms=g + 2)
                        return _end

                    # 8 pairs of k-tile slots: two adjacent 64-row-mode score
                    # slots per 128-mode stretch, halving tiling-mode switches
                    for ktp in range(KT // 2):
                        run_due(g + 1)
                        es = []
                        pe_t += SW_NS          # 128->64 mode switch drain
                        for kt in (2 * ktp, 2 * ktp + 1):
                            ksl = slice(kt * 128, (kt + 1) * 128)
                            if g >= 2:
                                # sc-ring WAR: this score waits exp[g-2]'s
                                # completion sem
                                pe_t = max(pe_t, exp_end[g - 2] + SEM_LAT)
                            sc = psum.tile([128, 2, 512], F32, tag="sc",
                                           bufs=2, name=f"sc_{qb}_{t}_{kt}")
                            nc.tensor.matmul(
                                sc[:, 0, :], kt_s[0:64, t, ksl],
                                qt_b[0:64, t, :],
                                start=True, stop=True, tile_position=(0, 0),
                            )
                            nc.tensor.matmul(
                                sc[:, 1, :], kt_s[64:128, t, ksl],
                                qt_b[64:128, t, :],
                                start=True, stop=True, tile_position=(64, 0),
                            )
                            pe_t += MM_NS
                            e = att.tile([128, 2, 512], F16, tag="exp", bufs=E_BUFS,
                                         name=f"e_{qb}_{t}_{kt}")
                            nc.scalar.activation(e[:], sc[:], EXP, scale=0.125)
                            act_t = max(act_t + ACT_SEM,
                                        pe_t + SC_DRAIN) + ACT_NS
                            exp_end.append(act_t)
                            es.append((kt, e))
                            g += 1
                        pe_t += SW_NS          # 64->128 mode switch drain
                        for kt, e in es:
                            tr = TRAIL_BLK if kt < 2 else TRAIL
                            pvq.append((tr, mk_pv(kt, e)))
                            if kt == KT - 1:
                                pvq.append((TRAIL_END, mk_end()))
                        while pvq and len(pvq) > pvq[0][0]:
                            pvq.pop(0)[1]()
                        run_spare(g - 1)

            while pvq:
                pvq.pop(0)[1]()
            # the PE idles ~3-6us here for the last normalize chain; without
            # activity HAM re-throttles and every tail matmul runs at 1.2GHz.
            # Fresh sc-tag psum: wps's proj-ring slot now belongs to a live
            # W_o partial accumulator.
            warm_tail = psum.tile([128, 128], F32, tag="sc", bufs=2,
                                  name="warm_tail")
            for _ in range(12):
                nc.tensor.matmul(warm_tail[:], warm_s[:], warm_s[:],
                                 start=True, stop=True)
            # pre-run W_o partial stages 2-5 on the freed score psum: each
            # 2-bank sc slot holds two 1-bank accumulators (subtile deps)
            sc_d0 = psum.tile([128, 2, 512], F32, tag="sc", bufs=2,
                              name="sc_d0")
            sc_d1 = psum.tile([128, 2, 512], F32, tag="sc", bufs=2,
                              name="sc_d1")
            ot_b3 = ot_tiles[SB - 1]
            for j, ps in ((2, sc_d0[:, 0, :]), (3, sc_d0[:, 1, :]),
                          (4, sc_d1[:, 0, :]), (5, sc_d1[:, 1, :])):
                si, dm = stages[j]
                st = (SB - 1) * 4 + si
                wo_ps[(si, dm)] = ps
                for t in range(NT - 1):
                    nc.tensor.matmul(
                        ps, ot_b3[:, t, si * 128:(si + 1) * 128],
                        wot_s[:, t, dm * 512:(dm + 1) * 512],
                        start=t == 0, stop=False,
                    )

            # ---- tail: W_o of the last q-block, split so the t=0..2 partial
            # accumulations overlap the (3,3) normalize-chain latency; only
            # the t=3 matmul + evacuation remain serialized behind it. The
            # lead partials are emitted BEFORE the deferred (3,3) chain so
            # they carry no dependency on its TT write. The "sc" pool has 3
            # slots: 3 partials lead, each final frees a slot for the next.
            for j in range(2):
                if (stages[j][0], stages[j][1]) not in wo_ps:
                    wo_partial(*stages[j])
            work.sort(key=lambda i: (i["ms"], i["dl"]))
            while work:
                it = work.pop(0)
                it["fn"]()
                if it["nxt"] is not None:
                    work.insert(0, it["nxt"])
            tail_dma[0] = True
            for j in range(len(stages)):
                wo_final(*stages[j])
                if j + 2 in (6, 7):
                    wo_partial(*stages[j + 2], tag="proj")
    nc.compile()
    return nc


_NC_CACHE = []


def _tile_x(x):
    # x: [S, D] -> x.T tiled [SB, 128, PKT, 512] with
    # tiled[sb, p, kt, s] = x.T[kt*128 + p, sb*512 + s]
    return np.ascontiguousarray(
        x.T.reshape(PKT, 128, SB, 512).transpose(2, 1, 0, 3).astype(np.float16))


def _tile_w(wt, nt, m):
    # wt: [D_in, m] (already transposed weight slice) -> [128, nt, m]
    return np.ascontiguousarray(
        wt.reshape(nt, 128, m).transpose(1, 0, 2).astype(np.float16))


def _tile_w_tmajor(wt):
    # wt: [D=1024, DP=512] -> [128, NT, PKT, 128] with
    # tiled[p, t, kt, j] = wt[kt*128 + p, t*128 + j]
    return np.ascontiguousarray(
        wt.reshape(PKT, 128, NT, 128).transpose(1, 2, 0, 3).astype(np.float16))


def kernel(**inputs):
    query = np.asarray(inputs["query"], dtype=np.float32)
    key = np.asarray(inputs["key"], dtype=np.float32)
    value = np.asarray(inputs["value"], dtype=np.float32)
    w_q = np.asarray(inputs["W_q"], dtype=np.float32)
    w_k = np.asarray(inputs["W_k"], dtype=np.float32)
    w_v = np.asarray(inputs["W_v"], dtype=np.float32)
    w_o = np.asarray(inputs["W_o"], dtype=np.float32)

    in_maps = []
    for c in range(8):
        b, hg = c // 2, c % 2
        dsl = slice(hg * DP, (hg + 1) * DP)
        in_maps.append({
            "xqt": _tile_x(query[b]),
            "xkt": _tile_x(key[b]),
            "xvt": _tile_x(value[b]),
            "wqt": _tile_w_tmajor(w_q[dsl, :].T),
            "wkt": _tile_w_tmajor(w_k[dsl, :].T),
            "wvt": _tile_w(w_v[dsl, :].T, PKT, DP),
            "wot": _tile_w(w_o[:, dsl].T, NT, D),
        })

    if not _NC_CACHE:
        _NC_CACHE.append(build_nc())
    nc = _NC_CACHE[0]
    res = run_bass_kernel_spmd(nc, in_maps, core_ids=list(range(8)),
                               **_RUN_KWARGS)
    _LAST_RESULT.clear()
    _LAST_RESULT.append(res)
    parts = [r["out"] for r in res.results]
    full = np.empty((B, S, D), dtype=np.float32)
    for b in range(B):
        full[b] = (parts[2 * b].astype(np.float32)
                   + parts[2 * b + 1].astype(np.float32))
    return full


# revision 23
# speedup vs baseline: 1.0001x; 1.0001x over previous
"""MultiHeadAttention Trainium2 Bass kernel, 8-core (batch x head-group) sharded.

Reference computation (B=4, S=2048, D=1024, H=16, d_k=64):
    Q = query @ W_q.T ; K = key @ W_k.T ; V = value @ W_v.T
    per head: attn = softmax(Q K^T / 8) @ V
    out = concat_heads(attn) @ W_o.T
Sharding: core c handles batch b = c // 2 and head-group hg = c % 2 (8 heads,
a 512-wide slice of the model dim). Core-pair partial fp16 outputs
(row-parallel W_o) are summed on the host while unsharding.

All matmul operands are fp16 (fp32 PSUM accumulation). Trace-derived budget
(per core): PE streams ~280us (scores 56us as concurrent dual-quadrant
pairs + PV 111us (M=65, the ones column rules out col-tiling) + QKV proj
83us + W_o 28us), ACT exp ~279us effective (256 ops of N=1024; PSUM
capacity forbids larger batches: sc 2x2 banks + proj 2 + ot 2 = 8). Steady
state runs at the ACT/PE joint floor (~1080ns/slot); the kernel spends its
effort on the edges: DMA-paced ramp, tiling-mode switch overhead, and the
tail.

Structure (measured 402.8us -> ~365us over this rework):
  prologue: HAM warm-up matmuls; input DMAs are chunked (<=0.5MB) across
  the sync/gpsimd rings (~120GB/s each) in score-need-order (xk before xv:
  scores gate the exp chain, V rides the PV trail); W_q/W_k ship t-major so
  head-pair slices land as 256KB units. The scalar ring carries ONLY the
  3 q-chain chunks, first: a DMA trigger whose queue-guard semaphore isn't
  ready head-of-line-blocks its engine queue, and the scalar queue carries
  the EXP chain.
  attention: 256 (qb, t, kt) score slots emitted in PAIRS (two adjacent
  64-row-mode score slots per 128-mode stretch of PV/proj work) to halve
  tiling-mode switch drains. EVERYTHING else (V projection halves, K.T
  tiles, Q.T blocks, W_o of finished blocks, softmax-normalize chains)
  flows through one deadline-tagged work queue paced by a cycle model that
  includes the exp->score semaphore latency (SEM_LAT) and switch drains
  (SW_NS), so run_spare fills the PE exactly up to each score pair's
  eligibility and the ACT chain never queues behind slack work. P.V
  matmuls trail their exp by a per-item pvq threshold (TRAIL/TRAIL_BLK/
  TRAIL_END) so block-boundary ot-ring WARs resolve off the critical path.
  tail: W_o(qb=3) partials pre-run on freed score-psum slots (two 1-bank
  accumulators per 2-bank sc slot), warm matmuls bridge the last normalize
  chain so HAM stays at 2.4GHz, and the final out-DMAs fan across all
  three rings.

Per-core dataflow (contraction always on the partition axis):
    K.T[d', s] = (W_k.T slice).T @ x_k.T    (d' on partitions)
    Q.T[d', s] likewise, projected per 512-wide q-block
    V[s, d']   = (x_v.T).T @ W_v.T          (natural layout, + ones column)
    S.T[k, q]  = (K_h.T).T @ Q_h.T          (two heads row-packed, K=64)
    expS.T     = exp(S.T / 8)               (one ACT op per k-tile, 1024 free)
    O.T+denom  = [V_h | 1].T @ expS.T       (M=65, accumulated over 16 k tiles)
    O.T norm   = O.T * (1/denom)            (DVE reciprocal + gpsimd broadcast,
                                             deferred off the block boundary)
    out[s, :]  = O.T.T @ W_o.T slice        (fp16 partial; host adds pairs)
"""
import sys

sys.path.insert(0, "/opt/trn_rl_repo")

import numpy as np

import concourse.bass as bass  # noqa: F401
import concourse.tile as tile
from concourse import bacc, mybir
from concourse.bass_utils import run_bass_kernel_spmd

F16 = mybir.dt.float16
F32 = mybir.dt.float32
EXP = mybir.ActivationFunctionType.Exp
COPY = mybir.ActivationFunctionType.Copy
MULT = mybir.AluOpType.mult

B, S, D = 4, 2048, 1024
H_PER_CORE = 8      # heads per core
DH = 64             # head dim
DP = 512            # per-core model-dim slice (8 heads x 64)
NT = 4              # d' tiles / head pairs per core
SB = 4              # 512-wide s/q blocks
KT = 16             # 128-wide k tiles
PKT = 8             # 128-wide contraction tiles for projections (D / 128)
VW = DH + 1         # V columns per head incl. ones column
N_WARM = 70         # HAM warm-up matmuls at kernel start
TRAIL = 7           # pvq items a P.V pair trails its exp (steady state)
TRAIL_BLK = 10      # trail for the first PVs of a block (ot-ring WAR room)
TRAIL_END = 5       # trail for mk_end (evacuate before the ot-ring wraps)
E_BUFS = 11         # exp-tile ring: >= TRAIL_BLK + 1

# cycle-model constants (ns) for opportunistic fill pacing
MM_NS = 219.0
ACT_NS = 1113.0
ACT_SEM = 58.0
SC_DRAIN = 170.0
SEM_LAT = 430.0     # exp-completion sem propagation to a waiting score MM
SW_NS = 90.0        # 64<->128 tiling-mode switch drain

_RUN_KWARGS = {}
_LAST_RESULT = []


def build_nc():
    nc = bacc.Bacc("TRN2", target_bir_lowering=False, debug=False)

    # activations pre-tiled on host: [sb, p, kt, 512], contiguous per partition
    xqt = nc.dram_tensor("xqt", [SB, 128, PKT, 512], F16, kind="ExternalInput")
    xkt = nc.dram_tensor("xkt", [SB, 128, PKT, 512], F16, kind="ExternalInput")
    xvt = nc.dram_tensor("xvt", [SB, 128, PKT, 512], F16, kind="ExternalInput")
    # W_q/W_k pre-tiled t-major: [p, t, kt, 128] so one head-pair's slice is
    # a 256KB contiguous chunk; W_v/W_o stay kt-major (consumed whole)
    wqt = nc.dram_tensor("wqt", [128, NT, PKT, 128], F16, kind="ExternalInput")
    wkt = nc.dram_tensor("wkt", [128, NT, PKT, 128], F16, kind="ExternalInput")
    wvt = nc.dram_tensor("wvt", [128, PKT, DP], F16, kind="ExternalInput")
    wot = nc.dram_tensor("wot", [128, NT, D], F16, kind="ExternalInput")
    out = nc.dram_tensor("out", [S, D], F16, kind="ExternalOutput")

    with tile.TileContext(nc) as tc:
        with tc.tile_pool(name="persist", bufs=1) as persist, \
             tc.tile_pool(name="psum", bufs=3, space="PSUM") as psum, \
             tc.tile_pool(name="att", bufs=2) as att:

            # ---- HAM warm-up: keep the PE busy while the first DMAs land ----
            warm_s = persist.tile([128, 128], F16)
            nc.vector.memset(warm_s[:], 1.0)
            wps = psum.tile([128, 128], F32, tag="proj", bufs=2, name="warm_ps")
            for _ in range(N_WARM):
                nc.tensor.matmul(wps[:], warm_s[:], warm_s[:],
                                 start=True, stop=True)

            # ---- persistent SBUF ----
            wk_s = persist.tile([128, NT, PKT, 128], F16)
            wv_s = persist.tile([128, PKT, DP], F16)
            wq_s = persist.tile([128, NT, PKT, 128], F16)
            wot_s = persist.tile([128, NT, D], F16)
            kt_s = persist.tile([128, NT, S], F16)          # K.T
            vext_s = persist.tile([128, KT, H_PER_CORE * VW], F16)  # [V_h | 1]
            xk_bs = [persist.tile([128, PKT, 512], F16, name=f"xk_{sb}")
                     for sb in range(SB)]

            # ones columns for the denominator rows (V part is written below)
            ones_f = persist.tile([128, KT, H_PER_CORE], F16)
            nc.vector.memset(ones_f[:], 1.0)
            nc.vector.tensor_copy(
                vext_s[:].rearrange("p k (h c) -> p k h c", c=VW)[:, :, :, DH:DH + 1],
                ones_f[:, :, :, None],
            )

            qt_tiles = {}

            # xv and xq activations share one 6-buffer tag: xq2/xq3 reuse the
            # buffers of xv0/xv1, whose V-projection reads finish ~100 slots
            # before those DMAs are triggered, so the buffer-reuse WAR waits
            # are always already satisfied (no cross-queue deadlock cycle)
            def xq_alloc(qb):
                xq_b = att.tile([128, PKT, 512], F16, tag="xbuf", bufs=6,
                                name=f"xq_{qb}")
                qt_tiles[qb] = (att.tile([128, NT, 512], F16, tag="qt", bufs=3,
                                         name=f"qt_{qb}"), xq_b)
                return xq_b

            def xq_load(qb, ring):
                xq_b = xq_alloc(qb)
                ring.dma_start(xq_b[:], xqt[qb])

            # ---- input DMAs. The scalar queue carries the EXP chain, and a
            # DMA trigger whose queue-guard semaphore isn't ready
            # head-of-line-blocks its engine's instruction queue — so the
            # scalar ring gets ONLY the 3 q-chain chunks, first, inside the
            # guard-free zone (~4 outstanding DMAs before guards appear),
            # and nothing else ever. sync and gpsimd (idle early) absorb
            # guard waits harmlessly. Chunks <=0.5MB in need-order; subtile
            # deps let each proj start as its chunk lands. ----
            xv_bs = [att.tile([128, PKT, 512], F16, tag="xbuf", bufs=6,
                              name=f"xv_{sb}") for sb in range(SB)]
            xq0_b = xq_alloc(0)
            xq1_b = xq_alloc(1)
            lo, hi = slice(0, PKT // 2), slice(PKT // 2, PKT)
            for dst, src in [(wq_s[:, 0], wqt[:, 0]),
                             (xq0_b[:, lo], xqt[0][:, lo]),
                             (xq0_b[:, hi], xqt[0][:, hi])]:
                nc.scalar.dma_start(dst, src)
            sync_chunks = [
                (wk_s[:, 0], wkt[:, 0]),            # 256KB: first kt proj
                (xk_bs[0][:, lo], xkt[0][:, lo]),
                (xk_bs[1][:, lo], xkt[1][:, lo]),   # kt(sb1) due slot 3
                (xk_bs[2][:, lo], xkt[2][:, lo]),   # scores gate the chain;
                (wv_s[:, lo], wvt[:, lo]),          # V rides the PV trail
                (xv_bs[0][:, lo], xvt[0][:, lo]),
                (xk_bs[3][:, lo], xkt[3][:, lo]),
                (wk_s[:, 1], wkt[:, 1]),            # head-pair 1 due slot 15
                (xv_bs[1][:, lo], xvt[1][:, lo]),
                (xv_bs[2][:, lo], xvt[2][:, lo]),
                (xv_bs[3][:, lo], xvt[3][:, lo]),
                (wk_s[:, 2:4], wkt[:, 2:4]),
                (xq1_b[:, lo], xqt[1][:, lo]),      # q-block 1 due slot 64
                (wot_s[:, 0:2], wot[:, 0:2]),       # W_o due ~slot 80
            ]
            gps_chunks = [
                (xk_bs[0][:, hi], xkt[0][:, hi]),
                (xk_bs[1][:, hi], xkt[1][:, hi]),
                (xk_bs[2][:, hi], xkt[2][:, hi]),
                (wv_s[:, hi], wvt[:, hi]),
                (xv_bs[0][:, hi], xvt[0][:, hi]),
                (xk_bs[3][:, hi], xkt[3][:, hi]),
                (wq_s[:, 1], wqt[:, 1]),
                (xv_bs[1][:, hi], xvt[1][:, hi]),
                (xv_bs[2][:, hi], xvt[2][:, hi]),
                (xv_bs[3][:, hi], xvt[3][:, hi]),
                (wq_s[:, 2:4], wqt[:, 2:4]),
                (xq1_b[:, hi], xqt[1][:, hi]),
                (wot_s[:, 2:4], wot[:, 2:4]),
            ]
            for dst, src in sync_chunks:
                nc.sync.dma_start(dst, src)
            for dst, src in gps_chunks:
                nc.gpsimd.dma_start(dst, src)

            # ---- work helpers (prologue + queue items) ----
            pr_ps = {}

            def ktproj(sb, t, half):
                key = ("kt", sb, t)
                if half == 0:
                    pr_ps[key] = psum.tile([128, 512], F32, tag="proj",
                                           bufs=2, name=f"psk_{sb}_{t}")
                ps = pr_ps[key]
                for kt in range(half * PKT // 2, (half + 1) * PKT // 2):
                    nc.tensor.matmul(
                        ps[:], wk_s[:, t, kt, :],
                        xk_bs[sb][:, kt, :],
                        start=kt == 0, stop=kt == PKT - 1,
                    )
                if half == 1:
                    nc.vector.tensor_copy(
                        kt_s[:, t, sb * 512:(sb + 1) * 512], ps[:])

            def vproj(st, half):
                sb, quarter = st // 4, st % 4
                key = ("v", st)
                if half == 0:
                    pr_ps[key] = psum.tile([128, 512], F32, tag="proj",
                                           bufs=2, name=f"psv_{st}")
                ps = pr_ps[key]
                for kt in range(half * PKT // 2, (half + 1) * PKT // 2):
                    nc.tensor.matmul(
                        ps[:], xv_bs[sb][:, kt, quarter * 128:(quarter + 1) * 128],
                        wv_s[:, kt, :],
                        start=kt == 0, stop=kt == PKT - 1,
                    )
                if half == 1:
                    nc.vector.tensor_copy(
                        vext_s[:, st, :].rearrange(
                            "p (h c) -> p h c", c=VW)[:, :, 0:DH],
                        ps[:].rearrange("p (h c) -> p h c", c=DH),
                    )

            def qt_proj(qb, t, half):
                qt_b, xq_b = qt_tiles[qb]
                key = ("q", qb, t)
                if half == 0:
                    pr_ps[key] = psum.tile([128, 512], F32, tag="proj",
                                           bufs=2, name=f"psq_{qb}_{t}")
                ps = pr_ps[key]
                for kt in range(half * PKT // 2, (half + 1) * PKT // 2):
                    nc.tensor.matmul(
                        ps[:], wq_s[:, t, kt, :],
                        xq_b[:, kt, :],
                        start=kt == 0, stop=kt == PKT - 1,
                    )
                if half == 1:
                    nc.vector.tensor_copy(qt_b[:, t, :], ps[:])

            ot_tiles = {}
            out_rr = [0]

            tail_dma = [False]

            def out_dma(dst, src):
                if tail_dma[0]:
                    r = (nc.sync, nc.gpsimd, nc.scalar)[out_rr[0] % 3]
                else:
                    r = nc.sync if out_rr[0] % 2 == 0 else nc.gpsimd
                r.dma_start(dst, src)
                out_rr[0] += 1

            def wo_stage(qb, si, dm, tail=False):
                ot_b = ot_tiles[qb]
                st = qb * 4 + si
                ssl = slice(si * 128, (si + 1) * 128)
                ps = psum.tile([128, 512], F32, tag="proj", bufs=2,
                               name=f"pso_{st}_{dm}")
                for t in range(NT):
                    nc.tensor.matmul(
                        ps[:], ot_b[:, t, ssl],
                        wot_s[:, t, dm * 512:(dm + 1) * 512],
                        start=t == 0, stop=t == NT - 1,
                    )
                ob = att.tile([128, 512], F16, tag="ob", bufs=5,
                              name=f"ob_{st}_{dm}")
                if tail and dm == 0:
                    # scalar engine is idle after the last EXP
                    nc.scalar.activation(ob[:], ps[:], COPY)
                else:
                    nc.vector.tensor_copy(ob[:], ps[:])
                out_dma(
                    out[st * 128:(st + 1) * 128, dm * 512:(dm + 1) * 512],
                    ob[:])

            def norm_chain(qb, t, otr_a, dn_a, otr_b, dn_b, ot_b):
                for nm, otr, dn, psl in (("a", otr_a, dn_a, slice(0, 64)),
                                         ("b", otr_b, dn_b, slice(64, 128))):
                    rd = att.tile([1, 512], F32, tag="rd", bufs=2,
                                  name=f"rd{nm}_{qb}_{t}")
                    nc.vector.reciprocal_approx_fast(rd[:], dn[:])
                    rb = att.tile([64, 512], F32, tag="rb", bufs=2,
                                  name=f"rb{nm}_{qb}_{t}")
                    nc.gpsimd.partition_broadcast(rb[:], rd[:])
                    nc.vector.tensor_tensor(
                        ot_b[psl, t, :], otr[:], rb[:], MULT)
                if (qb, t) == (SB - 1, NT - 2):
                    # ot_b[3][:, 0:3, :] is fully normalized now: the first
                    # two tail W_o partial stages can overlap the last slots
                    for si_dm in stages[:2]:
                        push(3 * MM_NS, partial(wo_partial, *si_dm),
                             dl=10**9, ms=0)

            stages = [(si, dm) for si in range(4) for dm in range(2)]
            wo_ps = {}

            def wo_partial(si, dm, tag="proj"):
                ot_b = ot_tiles[SB - 1]
                st = (SB - 1) * 4 + si
                ps = psum.tile([128, 512], F32, tag=tag, bufs=2,
                               name=f"pso_{st}_{dm}")
                wo_ps[(si, dm)] = ps
                for t in range(NT - 1):
                    nc.tensor.matmul(
                        ps[:], ot_b[:, t, si * 128:(si + 1) * 128],
                        wot_s[:, t, dm * 512:(dm + 1) * 512],
                        start=t == 0, stop=False,
                    )

            def wo_final(si, dm):
                ot_b = ot_tiles[SB - 1]
                st = (SB - 1) * 4 + si
                ps = wo_ps[(si, dm)]
                t = NT - 1
                nc.tensor.matmul(
                    ps[:], ot_b[:, t, si * 128:(si + 1) * 128],
                    wot_s[:, t, dm * 512:(dm + 1) * 512],
                    start=False, stop=True,
                )
                ob = att.tile([128, 512], F16, tag="ob", bufs=5,
                              name=f"ob_{st}_{dm}")
                if dm == 0:
                    nc.scalar.activation(ob[:], ps[:], COPY)
                else:
                    nc.vector.tensor_copy(ob[:], ps[:])
                out_dma(
                    out[st * 128:(st + 1) * 128, dm * 512:(dm + 1) * 512],
                    ob[:])

            # ---- prologue: just enough for the first score slots ----
            ktproj(0, 0, 0); ktproj(0, 0, 1)
            qt_proj(0, 0, 0); qt_proj(0, 0, 1)

            # ---- the work queue ----
            # dicts: c=PE cost ns, dl=deadline slot, ms=min slot. A projection
            # holds one sc-ring psum slot from its h0 until its h1: the "nxt"
            # link front-inserts h1 right after h0 runs, so at most one score
            # pair allocates between them and the 3-slot ring cannot wrap onto
            # the held psum.
            work = []

            def push(cost, fn, dl, ms=0, nxt=None):
                work.append({"c": cost, "dl": dl, "ms": ms, "fn": fn,
                             "nxt": nxt})

            def push2(cost, fn0, fn1, dl, ms=0):
                work.append({"c": cost, "dl": dl, "ms": ms, "fn": fn0,
                             "nxt": {"c": cost, "dl": -1e9, "ms": 0,
                                     "fn": fn1, "nxt": None}})

            from functools import partial

            for sb in range(1, SB):               # K.T head-pair 0
                push2(4 * MM_NS, partial(ktproj, sb, 0, 0),
                      partial(ktproj, sb, 0, 1), 4 * sb - 1)
            for st in range(KT):                  # V halves (DMA-ramp paced)
                push2(4 * MM_NS, partial(vproj, st, 0),
                      partial(vproj, st, 1), st + 6)
            for t in range(1, NT):                # K.T head-pairs 1-3
                for sb in range(SB):
                    push2(4 * MM_NS, partial(ktproj, sb, t, 0),
                          partial(ktproj, sb, t, 1), 16 * t + 4 * sb - 1)
                push2(4 * MM_NS, partial(qt_proj, 0, t, 0),
                      partial(qt_proj, 0, t, 1), 16 * t - 4)
            # xq2/xq3 DMAs reuse xv0/xv1 buffers: trigger them only after the
            # V projections that read those buffers have been emitted
            push(60, partial(xq_load, 2, nc.sync), 113, ms=36)
            push(60, partial(xq_load, 3, nc.sync), 177, ms=84)
            # Q.T for later q-blocks (qb 2/3 gated behind their xq_load)
            qt_ms = {1: 0, 2: 40, 3: 88}
            for qb in range(1, SB):
                for t in range(NT):
                    push2(4 * MM_NS, partial(qt_proj, qb, t, 0),
                          partial(qt_proj, qb, t, 1), 64 * qb + 16 * t - 2,
                          ms=qt_ms[qb])
            work.sort(key=lambda i: i["dl"])

            # ---- attention: 256 score slots at EXP pace, emitted in pairs ----
            pe_t, act_t = 0.0, 0.0
            exp_end = []
            pvq = []            # trailing P.V / block-end closures

            def run_due(g):
                nonlocal pe_t
                i = 0
                while i < len(work):
                    it = work[i]
                    if it["dl"] <= g + 1 and it["ms"] <= g:
                        work.pop(i)
                        it["fn"]()
                        pe_t += it["c"]
                        if it["nxt"] is not None:
                            work.insert(0, it["nxt"])
                            i = 0
                    else:
                        i += 1

            def run_spare(g):
                # fill the PE up to (but not past) the moment the NEXT pair's
                # first score becomes eligible, so that score never queues
                # behind slack work and the ACT chain never starves
                nonlocal pe_t
                tgt = (exp_end[g - 1] + SEM_LAT - 250.0) if g >= 1 else 0.0
                while work:
                    pick = None
                    for it in work:
                        if it["ms"] <= g:
                            pick = it
                            break
                    if pick is None or pe_t + pick["c"] > tgt:
                        break
                    work.remove(pick)
                    pick["fn"]()
                    pe_t += pick["c"]
                    if pick["nxt"] is not None:
                        work.insert(0, pick["nxt"])

            g = 0
            for qb in range(SB):
                if qb >= 1:                        # W_o of the previous block
                    wo_last = 24 + 4 * 7 if qb == SB - 1 else 28 + 5 * 7
                    for j, (si, dm) in enumerate(
                            (si, dm) for si in range(4) for dm in range(2)):
                        # ms must postdate the (qb-1, 3) norm chain, which
                        # trails its block end by TRAIL + dl=+8 slots; spread
                        # the stages over a wider window (the local PE
                        # oversubscription shows as +60ns/slot of exp pace)
                        dl = (64 * qb + 24 + 4 * j if qb == SB - 1
                              else 64 * qb + 28 + 5 * j)
                        push(4 * MM_NS + 120,
                             partial(wo_stage, qb - 1, si, dm),
                             dl=dl, ms=64 * qb + 16 + 2 * j)
                qt_b, _ = qt_tiles[qb]
                ot_b = att.tile([128, NT, 512], F16, tag="ot",
                                name=f"ot_{qb}")
                ot_tiles[qb] = ot_b
                for t in range(NT):
                    ota = psum.tile([65, 512], F32, tag="ot", bufs=2,
                                    name=f"ota_{qb}_{t}")
                    otb = psum.tile([65, 512], F32, tag="ot", bufs=2,
                                    name=f"otb_{qb}_{t}")
                    ha, hb = 2 * t, 2 * t + 1

                    def mk_pv(kt, e, ota=ota, otb=otb, ha=ha, hb=hb):
                        # first PVs of a block trail longer so the previous
                        # block's mk_end evacuations clear the ot ring WAR
                        def _pv():
                            nonlocal pe_t
                            nc.tensor.matmul(
                                ota[:], vext_s[:, kt, ha * VW:(ha + 1) * VW],
                                e[:, 0, :],
                                start=kt == 0, stop=kt == KT - 1,
                            )
                            nc.tensor.matmul(
                                otb[:], vext_s[:, kt, hb * VW:(hb + 1) * VW],
                                e[:, 1, :],
                                start=kt == 0, stop=kt == KT - 1,
                            )
                            pe_t += 2 * MM_NS
                        return _pv

                    def mk_end(qb=qb, t=t, ota=ota, otb=otb, ot_b=ot_b):
                        def _end():
                            if (qb, t) == (SB - 1, NT - 1):
                                # tail-critical: denominator copies + recips
                                # first (the serial chain), otr evacuations on
                                # the now-idle scalar engine in parallel
                                dns, otrs, rbs = [], [], []
                                for nm, ot_ps in (("a", ota), ("b", otb)):
                                    dn = att.tile([1, 512], F32, tag="dn",
                                                  bufs=2, name=f"dnt{nm}")
                                    nc.vector.tensor_copy(dn[:],
                                                          ot_ps[64:65, :])
                                    dns.append(dn)
                                for nm, ot_ps in (("a", ota), ("b", otb)):
                                    otr = att.tile([64, 512], F32, tag="otr",
                                                   bufs=2, name=f"otrt{nm}")
                                    nc.scalar.activation(otr[:],
                                                         ot_ps[0:64, :], COPY)
                                    otrs.append(otr)
                                for nm, dn in (("a", dns[0]), ("b", dns[1])):
                                    rd = att.tile([1, 512], F32, tag="rd",
                                                  bufs=2, name=f"rdt{nm}")
                                    nc.vector.reciprocal_approx_fast(rd[:],
                                                                     dn[:])
                                    rb = att.tile([64, 512], F32, tag="rb",
                                                  bufs=2, name=f"rbt{nm}")
                                    nc.gpsimd.partition_broadcast(rb[:],
                                                                  rd[:])
                                    rbs.append(rb)
                                nc.vector.tensor_tensor(
                                    ot_b[0:64, t, :], otrs[0][:], rbs[0][:],
                                    MULT)
                                nc.vector.tensor_tensor(
                                    ot_b[64:128, t, :], otrs[1][:], rbs[1][:],
                                    MULT)
                                return
                            evs = []
                            for nm, ot_ps in (("a", ota), ("b", otb)):
                                otr = att.tile([64, 512], F32, tag="otr",
                                               bufs=2, name=f"otr{nm}_{qb}_{t}")
                                nc.vector.tensor_copy(otr[:], ot_ps[0:64, :])
                                dn = att.tile([1, 512], F32, tag="dn", bufs=2,
                                              name=f"dn{nm}_{qb}_{t}")
                                nc.vector.tensor_copy(dn[:], ot_ps[64:65, :])
                                evs += [otr, dn]
                            push(0, partial(norm_chain, qb, t, evs[0], evs[1],
                                            evs[2], evs[3], ot_b),
                                 dl=g + 8, ms=g + 2)
                        return _end

                    # 8 pairs of k-tile slots: two adjacent 64-row-mode score
                    # slots per 128-mode stretch, halving tiling-mode switches
                    for ktp in range(KT // 2):
                        run_due(g + 1)
                        es = []
                        pe_t += SW_NS          # 128->64 mode switch drain
                        for kt in (2 * ktp, 2 * ktp + 1):
                            ksl = slice(kt * 128, (kt + 1) * 128)
                            if g >= 2:
                                # sc-ring WAR: this score waits exp[g-2]'s
                                # completion sem
                                pe_t = max(pe_t, exp_end[g - 2] + SEM_LAT)
                            sc = psum.tile([128, 2, 512], F32, tag="sc",
                                           bufs=2, name=f"sc_{qb}_{t}_{kt}")
                            nc.tensor.matmul(
                                sc[:, 0, :], kt_s[0:64, t, ksl],
                                qt_b[0:64, t, :],
                                start=True, stop=True, tile_position=(0, 0),
                            )
                            nc.tensor.matmul(
                                sc[:, 1, :], kt_s[64:128, t, ksl],
                                qt_b[64:128, t, :],
                                start=True, stop=True, tile_position=(64, 0),
                            )
                            pe_t += MM_NS
                            e = att.tile([128, 2, 512], F16, tag="exp", bufs=E_BUFS,
                                         name=f"e_{qb}_{t}_{kt}")
                            nc.scalar.activation(e[:], sc[:], EXP, scale=0.125)
                            act_t = max(act_t + ACT_SEM,
                                        pe_t + SC_DRAIN) + ACT_NS
                            exp_end.append(act_t)
                            es.append((kt, e))
                            g += 1
                        pe_t += SW_NS          # 64->128 mode switch drain
                        for kt, e in es:
                            tr = TRAIL_BLK if kt < 2 else TRAIL
                            pvq.append((tr, mk_pv(kt, e)))
                            if kt == KT - 1:
                                pvq.append((TRAIL_END, mk_end()))
                        while pvq and len(pvq) > pvq[0][0]:
                            pvq.pop(0)[1]()
                        run_spare(g - 1)

            while pvq:
                pvq.pop(0)[1]()
            # the PE idles ~3-6us here for the last normalize chain; without
            # activity HAM re-throttles and every tail matmul runs at 1.2GHz.
            # Fresh sc-tag psum: wps's proj-ring slot now belongs to a live
            # W_o partial accumulator.
            warm_tail = psum.tile([128, 128], F32, tag="sc", bufs=2,
                                  name="warm_tail")
            for _ in range(12):
                nc.tensor.matmul(warm_tail[:], warm_s[:], warm_s[:],
                                 start=True, stop=True)
            # pre-run W_o partial stages 2-5 on the freed score psum: each
            # 2-bank sc slot holds two 1-bank accumulators (subtile deps)
            sc_d0 = psum.tile([128, 2, 512], F32, tag="sc", bufs=2,
                              name="sc_d0")
            sc_d1 = psum.tile([128, 2, 512], F32, tag="sc", bufs=2,
                              name="sc_d1")
            ot_b3 = ot_tiles[SB - 1]
            for j, ps in ((2, sc_d0[:, 0, :]), (3, sc_d0[:, 1, :]),
                          (4, sc_d1[:, 0, :]), (5, sc_d1[:, 1, :])):
                si, dm = stages[j]
                st = (SB - 1) * 4 + si
                wo_ps[(si, dm)] = ps
                for t in range(NT - 1):
                    nc.tensor.matmul(
                        ps, ot_b3[:, t, si * 128:(si + 1) * 128],
                        wot_s[:, t, dm * 512:(dm + 1) * 512],
                        start=t == 0, stop=False,
                    )

            # ---- tail: W_o of the last q-block, split so the t=0..2 partial
            # accumulations overlap the (3,3) normalize-chain latency; only
            # the t=3 matmul + evacuation remain serialized behind it. The
            # lead partials are emitted BEFORE the deferred (3,3) chain so
            # they carry no dependency on its TT write. The "sc" pool has 3
            # slots: 3 partials lead, each final frees a slot for the next.
            for j in range(2):
                if (stages[j][0], stages[j][1]) not in wo_ps:
                    wo_partial(*stages[j])
            work.sort(key=lambda i: (i["ms"], i["dl"]))
            while work:
                it = work.pop(0)
                it["fn"]()
                if it["nxt"] is not None:
                    work.insert(0, it["nxt"])
            tail_dma[0] = True
            for j in range(len(stages)):
                wo_final(*stages[j])
                if j + 2 in (6, 7):
                    wo_partial(*stages[j + 2], tag="proj")
    nc.compile()
    return nc


_NC_CACHE = []


def _tile_x(x):
    # x: [S, D] -> x.T tiled [SB, 128, PKT, 512] with
    # tiled[sb, p, kt, s] = x.T[kt*128 + p, sb*512 + s]
    return np.ascontiguousarray(
        x.T.reshape(PKT, 128, SB, 512).transpose(2, 1, 0, 3).astype(np.float16))


def _tile_w(wt, nt, m):
    # wt: [D_in, m] (already transposed weight slice) -> [128, nt, m]
    return np.ascontiguousarray(
        wt.reshape(nt, 128, m).transpose(1, 0, 2).astype(np.float16))


def _tile_w_tmajor(wt):
    # wt: [D=1024, DP=512] -> [128, NT, PKT, 128] with
    # tiled[p, t, kt, j] = wt[kt*128 + p, t*128 + j]
    return np.ascontiguousarray(
        wt.reshape(PKT, 128, NT, 128).transpose(1, 2, 0, 3).astype(np.float16))


def kernel(**inputs):
    query = np.asarray(inputs["query"], dtype=np.float32)
    key = np.asarray(inputs["key"], dtype=np.float32)
    value = np.asarray(inputs["value"], dtype=np.float32)
    w_q = np.asarray(inputs["W_q"], dtype=np.float32)
    w_k = np.asarray(inputs["W_k"], dtype=np.float32)
    w_v = np.asarray(inputs["W_v"], dtype=np.float32)
    w_o = np.asarray(inputs["W_o"], dtype=np.float32)

    in_maps = []
    for c in range(8):
        b, hg = c // 2, c % 2
        dsl = slice(hg * DP, (hg + 1) * DP)
        in_maps.append({
            "xqt": _tile_x(query[b]),
            "xkt": _tile_x(key[b]),
            "xvt": _tile_x(value[b]),
            "wqt": _tile_w_tmajor(w_q[dsl, :].T),
            "wkt": _tile_w_tmajor(w_k[dsl, :].T),
            "wvt": _tile_w(w_v[dsl, :].T, PKT, DP),
            "wot": _tile_w(w_o[:, dsl].T, NT, D),
        })

    if not _NC_CACHE:
        _NC_CACHE.append(build_nc())
    nc = _NC_CACHE[0]
    res = run_bass_kernel_spmd(nc, in_maps, core_ids=list(range(8)),
                               **_RUN_KWARGS)
    _LAST_RESULT.clear()
    _LAST_RESULT.append(res)
    parts = [r["out"] for r in res.results]
    full = np.empty((B, S, D), dtype=np.float32)
    for b in range(B):
        full[b] = (parts[2 * b].astype(np.float32)
                   + parts[2 * b + 1].astype(np.float32))
    return full
